# revision 95
# baseline (speedup 1.0000x reference)
"""Trainium2 Bass kernel for nn_Attention_57406532878693 (pooling attention).

Math (per (b, h) slice; T=2048, N=128, K2=16):
    x      = hyp[:, b, h*128:(h+1)*128]                    # (T, N)
    m      = x.mean(0)                                     # (N,)
    gx     = tanh(x @ W_w.T + W_b)                         # (T, K2)
    gm     = tanh(Wm_w @ m + Wm_b)                         # (K2,)
    u      = Wh_w[0] * gm                                  # (K2,)
    l      = gx @ u + Wh_b                                 # (T,)
    p      = exp(l)          (no max-sub needed: |l| <= 4.25, tanh-bounded)
    c      = (p @ x) / p.sum()                             # (N,)
    out[b, h*128:(h+1)*128] = c

Sharding: data-parallel over B across 8 cores (4 batches per core).

v6 design (vs v4's 145us harness / 148.7us TimelineSim):
  - psB (the 128 accumulating mean matmuls, ~27us of PE) is gone: the
    PSUM->SBUF transpose evacuations carry accum_out, producing per-
    (head, j-block) partial t-sums for free; a tiny DVE fold + 4
    single-column matmuls reconstruct Wm@mean.
  - the 8 (batch, head-quad) slots are software-pipelined: slot k's
    emission order is [transposes+evac, psA+tanh, meanMM], z(k-1),
    psW(k-1) 1st half, [tanhc, u4, logits], psW(k-1) 2nd half +
    outscale(k-1), [memsets, exp], load(k+2).  psW of the previous slot
    fills the PE while slot k's gate chain round-trips through ACT/DVE,
    and the deferred z never head-of-line-blocks the next transposes.
  - consts ride in 2 packed DMAs (one f32, one bf16); nat loads are
    per-hq (4 tiles, 2-slot prefetch ring); each batch's output leaves
    via ONE partition-strided DMA on the idle GPSIMD queue (a DMA on
    the SP queue would block the in-order load stream behind its wait).
  - outscales run on DVE (ACT is the busier ALU); evac routing is
    10/16 to DVE (KB_EVDVE); a 14-deep warm-matmul chain on a memset
    scratch tile starts at t~=0.8us so the PE clock is warm when the
    first nat tile lands (also needed on HW: the HAM clock governor
    ignores PE-transpose cycles).
  - transposes read the nat tiles through a stride-2 bf16 bitcast view
    (a f32's high uint16 IS its truncated bf16), so the PE transpose
    runs at 1.0 cyc/row instead of f32r's 1.5 with no downcast pass;
    psT/xt stay bf16 (DVE evacuations get the 2x 16-bit mode), and psW
    still streams the full-f32 nat tiles so output precision holds.
  - the LAST slot's exp runs as two 8-chunk waves into two zero-padded
    p tiles (p_a/p_b) with partial Z sums, so its psW first half gates
    only on wave A and covers wave B + z on the PE; its tanh(j3) —
    which only gates logits c12-15 — is deferred between the exp waves
    so it never queues on ACT ahead of tanhc/exp-A (KB_EXP2).
TimelineSim single-shot (the harness-tracked metric; sim matched the
v4 baseline 148.7 vs 145.0): 148.7 (v4) -> 127.4 (psB cut) -> 118.1
-> 117.6 (tail exp split + deferred tanh) -> 116.1 (PSUM rebalance
pst=4/psc=1, viable only after the ham/psC decoupling).
Engine busy/core: PE 85.3us (65k transpose + 65k psA + 65k psW cycles),
DMA 93.7 (now the pacer: the bus is gap-free from 2.9us to 96.5us),
ACT ~64, DVE ~44.  Span = lead-in 2 + bus 94 + last-slot serial chain
~13 + drain 1.7; the chain (mean->gates->softmax->psW of the final
slot) and the DMA lead-in are the remaining slack.  NOTE: GPSIMD
cannot access PSUM on real HW (BIR verifier) — the sim doesn't know
and will falsely promise wins from Pool evacuations/out-scales.
Numerics: rel err 2.13e-4 vs reference (bf16-trunc gate path is ~100x
inside the 2e-2 budget; f32r weighted sum).
"""

import os
import numpy as np

T, B, D = 2048, 32, 1024
H, N, K2 = 8, 128, 16
NCORES = 8
BL = B // NCORES          # 4 batches per core
TC = T // 128             # 16 t-chunks of 128
NQ = 4                    # nat tiles per (batch, head-quad); each holds 512 t
QW = 4 * N                # 512 cols per head-quad

LAST_RESULT = {}          # exec_time_ns etc. for test harness introspection


def _build(nc, tile, mybir, bass, whb_val, repeat=1, loop_n=0):
    f32 = mybir.dt.float32
    f32r = mybir.dt.float32r
    bf16 = mybir.dt.bfloat16
    AF = mybir.ActivationFunctionType

    hyp_s = nc.dram_tensor("hyp_s", [T, BL, D], f32, kind="ExternalInput").ap()
    cpack_d = nc.dram_tensor("cpack", [128, 233], f32,
                             kind="ExternalInput").ap()
    wgzm_d = nc.dram_tensor("wgzm", [N, 192], bf16, kind="ExternalInput").ap()
    out_s = nc.dram_tensor("out_s", [BL, D], f32, kind="ExternalOutput").ap()

    def r(ap):
        return ap.bitcast(f32r)

    with tile.TileContext(nc) as tc:
        from contextlib import ExitStack

        with ExitStack() as ctx:
            natf_b = int(os.environ.get("KB_NATF", "14"))
            xt_b = int(os.environ.get("KB_XT", "8"))
            g_b = int(os.environ.get("KB_G", "4"))
            # psc=1 became safe (and pst=4 profitable) once the warm
            # matmuls moved off psC and the tail restructure made all psC
            # column ranges interleave disjointly across consecutive slots
            pst_b = int(os.environ.get("KB_PST", "4"))
            psa_b = int(os.environ.get("KB_PSA", "2"))
            psc_b = int(os.environ.get("KB_PSC", "1"))
            psw_b = int(os.environ.get("KB_PSW", "1"))
            # number of the 16 per-slot PSUM->SBUF evacuations routed to
            # DVE (the rest go to ACT, which also carries tanh/exp/out)
            evdve = int(os.environ.get("KB_EVDVE", "8"))
            evdve_tail = int(os.environ.get("KB_EVDVET", "9"))
            # psW(k-1) chunk split around logits(k): first part covers the
            # u4 round-trip, the rest covers the exp round-trip
            psplit = int(os.environ.get("KB_PSPLIT", "8"))
            # NOTE: GPSIMD cannot access PSUM on real HW (neuronxcc BIR
            # verifier rejects it; the TimelineSim cost model doesn't know
            # and predicts a ~1.7us win from Pool out-scales).  All PSUM
            # readers must stay on DVE/ACT.
            osplit_mid = os.environ.get("KB_OSPLITM", "0") == "1"
            onpool = int(os.environ.get("KB_ONPOOL", "4"))
            evpool = os.environ.get("KB_EVPOOL", "0") == "1"
            # split the LAST slot's exp into two 8-chunk waves writing two
            # zero-padded p tiles, so its psW chunks c<8 gate only on wave
            # A and cover wave B + z on the PE (mid slots don't need it and
            # the extra 4 exp instrs would cost more ACT than they save)
            exp2_tail = os.environ.get("KB_EXP2", "1") == "1"
            cpool = ctx.enter_context(tc.tile_pool(name="consts", bufs=1))
            natf_pool = ctx.enter_context(tc.tile_pool(name="natf", bufs=natf_b))
            xt_pool = ctx.enter_context(tc.tile_pool(name="xt", bufs=xt_b))
            g_pool = ctx.enter_context(tc.tile_pool(name="g", bufs=g_b))
            sm_pool = ctx.enter_context(tc.tile_pool(name="small", bufs=4))
            out_pool = ctx.enter_context(tc.tile_pool(name="outp", bufs=1))
            pst_pool = ctx.enter_context(
                tc.tile_pool(name="pst", bufs=pst_b, space="PSUM"))
            psa_pool = ctx.enter_context(
                tc.tile_pool(name="psa", bufs=psa_b, space="PSUM"))
            psc_pool = ctx.enter_context(
                tc.tile_pool(name="psc", bufs=psc_b, space="PSUM"))
            psw_pool = ctx.enter_context(
                tc.tile_pool(name="psw", bufs=psw_b, space="PSUM"))

            cp = cpool.tile([128, 233], f32, tag="cpack")
            nc.sync.dma_start(r(cp[:]), r(cpack_d))
            wgzm = cpool.tile([N, 192], bf16, tag="wgzm")
            nc.sync.dma_start(wgzm[:], wgzm_d)
            ident = cp[:, 0:128]
            wbc = cp[:, 128:129]
            wmbc = cp[:, 129:130]
            whwm = cp[:, 130:134]
            ones_c = cp[:, 134:136]
            zmask = cp[:, 136:233]
            wgz = wgzm[:, 0:32]
            wmz = wgzm[:, 32:64]
            identb = wgzm[:, 64:192]
            # bf16 transposes read the HIGH 2 bytes of each f32 nat element
            # (little-endian: a f32's high uint16 IS its truncated bf16), so
            # the PE transpose runs at 1.0 cyc/row instead of f32r's 1.5 —
            # no downcast pass anywhere.  psW still streams the full-f32
            # nat tiles, so output precision is unchanged.
            tbf16 = os.environ.get("KB_TBF16", "1") == "1"
            whb_c = cpool.tile([128, 1], f32, tag="whb")
            nc.gpsimd.memset(whb_c[:], float(whb_val))

            # PE transposes don't register as PE-busy for the HAM clock
            # governor, so a transpose-heavy stretch can drop the PE to
            # 1.2GHz.  KB_WARM sprinkles tiny real matmuls to keep the
            # activity window fed.
            warm = os.environ.get("KB_WARM", "1") == "1"
            warm_n = int(os.environ.get("KB_WARMN", "14"))
            wide_early = int(os.environ.get("KB_WIDE", "2"))
            # scratch region in the psw bank for warm matmuls: partition 0,
            # cols 256:512 — disjoint from every outscale read slice
            # ([32q, 128q:128(q+1)]), and strictly PE-ordered vs psW writes.
            warm_ps = psw_pool.tile([128, 512], f32, tag="psw")

            def ham_warm(width=2):
                if warm:
                    nc.tensor.matmul(warm_ps[0:1, 256:256 + width],
                                     r(ones_c[:, 0:1]),
                                     r(ident[0:128, 0:width]),
                                     start=True, stop=True,
                                     skip_group_check=True)

            if warm:
                # pre-loop clock warm-up: chained dummies on a never-written
                # scratch tile (contents irrelevant), so they have NO input
                # dependencies and start right after the preamble barrier —
                # the PE is at full clock by the time the first nat tile
                # lands.
                wsc = cpool.tile([128, 128], f32, tag="wsc")
                nc.gpsimd.memset(wsc[:], 1.0)
                for _ in range(warm_n):
                    nc.tensor.matmul(warm_ps[0:1, 0:128],
                                     r(wsc[:, 0:1]), r(wsc[:]),
                                     start=True, stop=True,
                                     skip_group_check=True)

            out_sb = out_pool.tile([97, BL * D // 4], f32, tag="out")

            def load_nat_hq(b, hq, eng=None):
                tiles = []
                for j in range(NQ):
                    t0 = j * 4 * 128
                    nt = natf_pool.tile([128, 4 * QW], f32, tag="natf")
                    src = hyp_s[t0:t0 + 4 * 128, b:b + 1,
                                hq * QW:(hq + 1) * QW].rearrange(
                        "(c p) one d -> p c (one d)", p=128)
                    (eng or nc.sync).dma_start(
                        r(nt[:].rearrange("p (c d) -> p c d", c=4)),
                        r(src))
                    tiles.append(nt)
                return tiles

            def phase1a(natf, ev_n, wide=2, defer_tanh=False):
                """transposes+evac (with mean accum), psA+tanh, mean MMs."""
                psC = psc_pool.tile([128, 72], f32, tag="psc")
                msum = sm_pool.tile([128, 16], f32, tag="msum")
                xts = []
                ei = 0
                st_defer = None
                for j in range(NQ):
                    nf = natf[j]
                    xt = xt_pool.tile([128, 4 * QW], bf16, tag="xt")
                    for q in range(4):
                        psT = pst_pool.tile([128, 512],
                                            bf16 if tbf16 else f32,
                                            tag="pst")
                        ham_warm(wide)
                        for c in range(4):
                            if tbf16:
                                lo = 2 * (512 * c + 128 * q) + 1
                                nc.tensor.transpose(
                                    psT[:, 128 * c:128 * (c + 1)],
                                    nf[:].bitcast(bf16)[:, lo:lo + 255:2],
                                    identb[:])
                            else:
                                nc.tensor.transpose(
                                    r(psT[:, 128 * c:128 * (c + 1)]),
                                    r(nf[:, 512 * c + 128 * q:
                                         512 * c + 128 * (q + 1)]),
                                    r(ident[:]))
                        dst = xt[:, 512 * q:512 * (q + 1)]
                        acc = msum[:, 4 * q + j:4 * q + j + 1]
                        use_dve = (((ei + 1) * ev_n) // 16
                                   > (ei * ev_n) // 16)
                        use_pool = (not use_dve and evpool
                                    and (ei % 8) == 7)
                        ei += 1
                        if use_dve:
                            nc.vector.tensor_scalar(
                                dst, psT[:], 1.0, 0.0,
                                op0=mybir.AluOpType.mult,
                                op1=mybir.AluOpType.add,
                                accum_out=acc)
                        elif use_pool:
                            nc.gpsimd.tensor_scalar(
                                dst, psT[:], 1.0, 0.0,
                                op0=mybir.AluOpType.mult,
                                op1=mybir.AluOpType.add,
                                accum_out=acc)
                        else:
                            nc.scalar.activation(dst, psT[:], AF.Copy,
                                                 accum_out=acc)
                    xts.append(xt)

                g_sb = g_pool.tile([128, T], bf16, tag="g")
                for j in range(NQ):
                    psA = psa_pool.tile([128, 512], f32, tag="psa")
                    for q in range(4):
                        rhs = xts[j][:, 512 * q:512 * (q + 1)]
                        nc.tensor.matmul(
                            psA[32 * q:32 * q + 32, :], wgz[:], rhs,
                            start=True, stop=True,
                            tile_position=(0, 32 * q),
                            skip_group_check=True)
                    if defer_tanh and j == NQ - 1:
                        # last slot: tanh(j3) only gates logits c12-15, but
                        # in ACT's in-order queue it would sit AHEAD of
                        # tanhc/exp-wave-A and delay the psW-a path.  Defer
                        # its emission into phase1b, between the exp waves.
                        st_defer = (g_sb[:, 512 * j:512 * (j + 1)], psA)
                    else:
                        nc.scalar.activation(
                            g_sb[:, 512 * j:512 * (j + 1)], psA[:],
                            AF.Tanh, bias=wbc[:])

                # mean-gate path: fold the 4 j-partials per head, then 4
                # single-column matmuls put Wm@mean at partitions 32q+k
                # of psC[:, 68]
                m4 = sm_pool.tile([128, 4], bf16, tag="m4")
                with nc.allow_low_precision(
                        reason="4-elem j-partial fold; |sums|~45, bf16 ok"):
                    nc.vector.tensor_reduce(
                        m4[:], msum[:].rearrange("p (q j) -> p q j", q=4),
                        axis=mybir.AxisListType.X,
                        op=mybir.AluOpType.add)
                for q in range(4):
                    nc.tensor.matmul(
                        psC[32 * q:32 * q + 32, 68:69], wmz[:],
                        m4[:, q:q + 1],
                        start=True, stop=True,
                        tile_position=(0, 32 * q),
                        skip_group_check=True)
                return {"psC": psC, "g_sb": g_sb, "defer": st_defer}

            def zfin(st, z_home=None):
                """z matmul(s) + reciprocal (deferred to the next iteration
                so it never head-of-line blocks the next slot's transposes).
                z_home overrides the PSUM scratch columns — the deferred
                tail-first slot uses a LIVE slot's psC spare cols, since its
                own psC bank has been recycled by then."""
                bank = st["psC"] if z_home is None else z_home
                cs_ps = bank[0:128, 64:66]
                zz_ps = bank[0:97, 69:71]
                # Z without exp accum_out (saves 187ns of ACT accumulator
                # readout per exp instr on the serial ACT chain): a column-
                # sum matmul (p stationary, ones moving) puts per-column
                # sums at partitions 32q+c, a tiny DVE evac moves them to
                # SBUF, and a constant block-diagonal mask matmul lands
                # Z_q directly on partition 32q.  Emitted HERE (deferred):
                # the PE reaches this point with the exps long done, so
                # nothing head-of-line-blocks (a DVE reduce right after
                # the exps would stall the next slot's evacuations).
                prs = st["prs"]
                z_ps = cs_ps[0:97, 0:2]
                for i, pr in enumerate(prs):
                    nc.tensor.matmul(z_ps, r(pr[:]), r(ones_c[:]),
                                     start=(i == 0),
                                     stop=(i == len(prs) - 1),
                                     skip_group_check=True)
                zi_sb = sm_pool.tile([97, 1], f32, tag="zi_sb")
                nc.vector.reciprocal(zi_sb[:], z_ps[0:97, 0:1])
                st["zi_sb"] = zi_sb

            def phase1b(st, exp2=False):
                """tanhc, u4, logits, memsets, exp."""
                psC, g_sb = st["psC"], st["g_sb"]
                tanhc = sm_pool.tile([128, 1], f32, tag="tanhc")
                nc.scalar.activation(tanhc[:], psC[:, 68:69], AF.Tanh,
                                     bias=wmbc[:])
                u4 = sm_pool.tile([128, 4], bf16, tag="u4")
                nc.vector.tensor_mul(
                    u4[:], whwm[:], tanhc[:].broadcast_to([128, 4]))

                # logits t-major: l[t, q] = sum_p g[p, t] * U4[p, q]
                c_wave1 = TC // 2 if exp2 else TC
                for c in range(c_wave1):
                    nc.tensor.matmul(
                        psC[:, 4 * c:4 * c + 4],
                        g_sb[:, 128 * c:128 * (c + 1)], u4[:],
                        start=True, stop=True, skip_group_check=True)

                lview = psC[:, 0:64].rearrange("p (c q) -> p q c", q=4)
                if exp2:
                    # two zero-padded p tiles: p_a holds exp(l) for chunks
                    # c<8, p_b for c>=8 (zeros elsewhere).  psW chunk c<8
                    # then only depends on the first exp wave, so the final
                    # slot's psW first half overlaps the second wave.
                    p_a = sm_pool.tile([128, 144], f32, tag="p_a")
                    p_b = sm_pool.tile([128, 144], f32, tag="p_b")
                    pr_a = sm_pool.tile([128, 97], f32, tag="pr_a")
                    pr_b = sm_pool.tile([128, 97], f32, tag="pr_b")
                    nc.gpsimd.memset(p_a[:], 0.0)
                    nc.gpsimd.memset(p_b[:], 0.0)
                    nc.gpsimd.memset(pr_a[:], 1.0)
                    nc.gpsimd.memset(pr_b[:], 1.0)
                    hc = TC // 2
                    with nc.allow_low_precision(
                            reason="f32r accum is fp32-width"):
                        for h, (p_x, pr_x) in enumerate(
                                ((p_a, pr_a), (p_b, pr_b))):
                            # each wave reads an exactly-bounded psC view
                            # (cols 32h:32h+32), so wave A's strided read
                            # can never false-share with wave B's logits
                            # writes under span-based dependency tracking
                            lv_h = psC[:, 32 * h:32 * h + 32].rearrange(
                                "p (c q) -> p q c", q=4)
                            for q in range(4):
                                nc.scalar.activation(
                                    r(p_x[:, 32 * q + h * hc:
                                          32 * q + h * hc + hc]
                                      .unsqueeze(1)),
                                    lv_h[:, q:q + 1, :],
                                    AF.Exp, bias=whb_c[:],
                                    accum_out=r(pr_x[:, 32 * q:32 * q + 1]))
                            if h == 0:
                                # deferred tanh(j3) + second logits wave —
                                # emitted after exp wave A so the ACT queue
                                # reaches wave A without the tanh in front
                                if st.get("defer") is not None:
                                    g_dst, psA_d = st["defer"]
                                    nc.scalar.activation(
                                        g_dst, psA_d[:], AF.Tanh,
                                        bias=wbc[:])
                                for c in range(TC // 2, TC):
                                    nc.tensor.matmul(
                                        psC[:, 4 * c:4 * c + 4],
                                        g_sb[:, 128 * c:128 * (c + 1)],
                                        u4[:], start=True, stop=True,
                                        skip_group_check=True)
                    st["ps"] = (p_a, p_b)
                    st["nps"] = 2
                    st["prs"] = (pr_a, pr_b)
                else:
                    p_quad = sm_pool.tile([128, 144], f32, tag="p_quad")
                    pr_quad = sm_pool.tile([128, 97], f32, tag="pr_quad")
                    nc.gpsimd.memset(p_quad[:], 0.0)
                    nc.gpsimd.memset(pr_quad[:], 1.0)
                    with nc.allow_low_precision(
                            reason="f32r accum is fp32-width"):
                        for q in range(4):
                            nc.scalar.activation(
                                r(p_quad[:, 32 * q:32 * q + TC]
                                  .unsqueeze(1)),
                                lview[:, q:q + 1, :],
                                AF.Exp, bias=whb_c[:],
                                accum_out=r(pr_quad[:, 32 * q:32 * q + 1]))
                    st["ps"] = (p_quad, p_quad)
                    st["nps"] = 1
                    st["prs"] = (pr_quad,)

            def psw_mm(natf, st, c0, c1):
                """psW weighted-sum chunks [c0, c1)."""
                if c0 == 0:
                    psw_t = psw_pool.tile([128, 512], f32, tag="psw")
                    st["psW"] = psw_t
                psW = st["psW"]
                p_lo, p_hi = st["ps"]
                for c in range(c0, c1):
                    j, cl = c // 4, c % 4
                    p_x = p_lo if c < TC // 2 else p_hi
                    rhs = natf[j][:, 512 * cl:512 * (cl + 1)]
                    nc.tensor.matmul(psW[:], r(p_x[:, c:c + 128]),
                                     r(rhs),
                                     start=(c == 0), stop=(c == TC - 1),
                                     skip_group_check=True)

            def phase2fin(b, hq, st, split_eng=False, do_dma=None):
                """out scale (+ per-batch out DMA)."""
                psW, zi_sb = st["psW"], st["zi_sb"]
                for q in range(4):
                    col = b * (D // 4) + hq * N
                    if split_eng and q >= 2:
                        # at the kernel tail ACT is idle once exp-B drains
                        # (post exp2/defer restructure), so it can take two
                        # of the final out-scales in parallel with DVE and
                        # the out DMA launches ~0.7us sooner.  (GPSIMD
                        # cannot read PSUM, so Pool is not an option here.)
                        nc.scalar.activation(
                            out_sb[32 * q:32 * q + 1, col:col + N],
                            psW[32 * q:32 * q + 1, q * N:(q + 1) * N],
                            AF.Copy, bias=0.0,
                            scale=zi_sb[32 * q:32 * q + 1, 0:1])
                    else:
                        nc.vector.tensor_scalar(
                            out_sb[32 * q:32 * q + 1, col:col + N],
                            psW[32 * q:32 * q + 1, q * N:(q + 1) * N],
                            zi_sb[32 * q:32 * q + 1, 0:1], None,
                            op0=mybir.AluOpType.mult)
                if (hq == 1) if do_dma is None else do_dma:
                    # batch b fully scaled -> stream its row out now.  One
                    # partition-strided DMA, issued from the idle GPSIMD
                    # queue so it never blocks the SP load queue's head.
                    nc.gpsimd.dma_start(
                        out_s[b:b + 1, :].rearrange(
                            "one (j q n) -> one q j n", q=4, n=N),
                        out_sb[0:97:32,
                               b * (D // 4):(b + 1) * (D // 4)]
                        .rearrange("p (j n) -> p j n", n=N))

            def run_schedule(slots):
                n = len(slots)
                nat = {0: load_nat_hq(*slots[0])}
                if n > 1:
                    nat[1] = load_nat_hq(*slots[1])
                prev = None
                for i, (b, hq) in enumerate(slots):
                    st = phase1a(nat[i],
                                 evdve_tail if i == n - 1 else evdve,
                                 wide=wide_early if i < 2 else 2,
                                 defer_tanh=exp2_tail and i == n - 1)
                    if prev is not None:
                        pi, pst_ = prev
                        zfin(pst_)
                        psw_mm(nat[pi], pst_, 0, psplit)
                    phase1b(st, exp2=exp2_tail and i == n - 1)
                    if prev is not None:
                        pi, pst_ = prev
                        psw_mm(nat[pi], pst_, psplit, TC)
                        phase2fin(slots[pi][0], slots[pi][1], pst_,
                                  split_eng=osplit_mid)
                        del nat[pi]
                    prev = (i, st)
                    if i + 2 < n:
                        nat[i + 2] = load_nat_hq(*slots[i + 2])
                pi, pst_ = prev
                if pst_["nps"] == 2:
                    # final slot: psW's first half only needs the first exp
                    # wave; it covers the second wave + z on the PE
                    psw_mm(nat[pi], pst_, 0, TC // 2)
                    zfin(pst_)
                    psw_mm(nat[pi], pst_, TC // 2, TC)
                else:
                    zfin(pst_)
                    psw_mm(nat[pi], pst_, 0, TC)
                phase2fin(slots[pi][0], slots[pi][1], pst_,
                          split_eng=os.environ.get("KB_OSPLIT", "0") == "1")

            def run_schedule_tailfirst(slots):
                """The LAST slot's tiles load first and its whole gate
                pipeline runs during the DMA-starved head; its z/psW/out
                close out mid-kernel where bus-pacing hides the PE work.
                The kernel then ends on slots[n-2], whose chain is the only
                remaining post-last-byte work."""
                n = len(slots)
                last = n - 1
                nat = {last: load_nat_hq(*slots[last])}
                nat[0] = load_nat_hq(*slots[0])
                st_last = phase1a(nat[last], evdve, wide=wide_early)
                phase1b(st_last)
                nat[1] = load_nat_hq(*slots[1])
                prev = None
                for i in range(n - 1):
                    st = phase1a(nat[i],
                                 evdve_tail if i == n - 2 else evdve,
                                 wide=wide_early if i < 1 else 2)
                    if prev is not None:
                        pi, pst_ = prev
                        zfin(pst_)
                        psw_mm(nat[pi], pst_, 0, psplit)
                    phase1b(st)
                    if prev is not None:
                        pi, pst_ = prev
                        psw_mm(nat[pi], pst_, psplit, TC)
                        phase2fin(slots[pi][0], slots[pi][1], pst_,
                                  split_eng=osplit_mid)
                        del nat[pi]
                    prev = (i, st)
                    if i == 2:
                        # deferred close-out of the tail-first slot; its z
                        # lands in the LIVE slot's psC spare cols (its own
                        # bank has been recycled)
                        zfin(st_last, z_home=st["psC"][0:97, 69:71])
                        psw_mm(nat[last], st_last, 0, TC)
                        phase2fin(slots[last][0], slots[last][1], st_last,
                                  do_dma=False)
                        del nat[last]
                    if i + 2 < n - 1:
                        nat[i + 2] = load_nat_hq(*slots[i + 2])
                pi, pst_ = prev
                zfin(pst_)
                psw_mm(nat[pi], pst_, 0, TC)
                # slots[n-2] is (b3, hq0); its batch-mate (b3, hq1) was the
                # tail-first slot, already scaled -> DMA batch 3 now
                phase2fin(slots[pi][0], slots[pi][1], pst_, do_dma=True,
                          split_eng=os.environ.get("KB_OSPLIT", "0") == "1")

            base_slots = [(b, hq) for b in range(BL) for hq in range(2)]
            tailfirst = os.environ.get("KB_TAILFIRST", "0") == "1"
            if loop_n:
                with tc.For_i(0, loop_n, 1):
                    run_schedule(base_slots)
            elif tailfirst and repeat == 1:
                run_schedule_tailfirst(base_slots)
            else:
                run_schedule(base_slots * repeat)
    return nc


def _consts(inputs):
    import ml_dtypes
    W_w = np.asarray(inputs["W_w"], dtype=np.float32)      # (K2, N)
    W_b = np.asarray(inputs["W_b"], dtype=np.float32)      # (K2,)
    Wm_w = np.asarray(inputs["Wm_w"], dtype=np.float32)    # (K2, N)
    Wm_b = np.asarray(inputs["Wm_b"], dtype=np.float32)    # (K2,)
    Wh_w = np.asarray(inputs["Wh_w"], dtype=np.float32)    # (1, K2)

    bf = ml_dtypes.bfloat16
    wgz = np.zeros((N, 32), np.float32)
    wgz[:, 0:K2] = W_w.T
    wmz = np.zeros((N, 32), np.float32)
    wmz[:, 0:K2] = Wm_w.T / T
    wgzm = np.concatenate(
        [wgz, wmz, np.eye(N, dtype=np.float32)], axis=1).astype(bf)

    cpack = np.zeros((128, 233), np.float32)
    cpack[:, 0:128] = np.eye(128, dtype=np.float32)
    for q in range(4):
        cpack[32 * q:32 * q + K2, 128] = W_b
        cpack[32 * q:32 * q + K2, 129] = Wm_b
        cpack[32 * q:32 * q + K2, 130 + q] = Wh_w[0]
    cpack[:, 134:136] = 1.0
    # z block-diagonal mask (cols 136:233 = [128, 97]): row 32q+c (c<16)
    # has a 1 in col 32q, so  zmask.T @ colsums  lands Z_q on partition 32q
    for q in range(4):
        cpack[32 * q:32 * q + K2, 136 + 32 * q] = 1.0
    return {"cpack": cpack, "wgzm": wgzm}


def kernel(**inputs):
    import concourse.bass as bass
    import concourse.bacc as bacc
    import concourse.tile as tile
    import concourse.mybir as mybir
    from concourse import bass_utils

    hyp = np.ascontiguousarray(np.asarray(inputs["hyp"], dtype=np.float32))
    Wh_b = np.asarray(inputs["Wh_b"], dtype=np.float32)    # (1,)

    nc = bacc.Bacc("TRN2", target_bir_lowering=False, debug=False)
    _build(nc, tile, mybir, bass, float(Wh_b.reshape(-1)[0]))
    nc.compile()

    consts = _consts(inputs)
    in_maps = []
    for j in range(NCORES):
        m = {"hyp_s": np.ascontiguousarray(hyp[:, j * BL:(j + 1) * BL, :])}
        m.update(consts)
        in_maps.append(m)

    trace = os.environ.get("BASS_KERNEL_TRACE", "0") == "1"
    res = bass_utils.run_bass_kernel_spmd(
        nc, in_maps, core_ids=list(range(NCORES)), trace=trace)

    LAST_RESULT.clear()
    LAST_RESULT["exec_time_ns"] = res.exec_time_ns
    LAST_RESULT["trace"] = (res.instructions_and_trace[1]
                            if res.instructions_and_trace else None)
    LAST_RESULT["profile_json"] = res.profile_json

    out = np.concatenate([res.results[j]["out_s"] for j in range(NCORES)],
                         axis=0)
    return out.astype(np.float32)


# revision 96
# speedup vs baseline: 1.0006x; 1.0006x over previous
"""Trainium2 Bass kernel for nn_Attention_57406532878693 (pooling attention).

Math (per (b, h) slice; T=2048, N=128, K2=16):
    x      = hyp[:, b, h*128:(h+1)*128]                    # (T, N)
    m      = x.mean(0)                                     # (N,)
    gx     = tanh(x @ W_w.T + W_b)                         # (T, K2)
    gm     = tanh(Wm_w @ m + Wm_b)                         # (K2,)
    u      = Wh_w[0] * gm                                  # (K2,)
    l      = gx @ u + Wh_b                                 # (T,)
    p      = exp(l)          (no max-sub needed: |l| <= 4.25, tanh-bounded)
    c      = (p @ x) / p.sum()                             # (N,)
    out[b, h*128:(h+1)*128] = c

Sharding: data-parallel over B across 8 cores (4 batches per core).

v6 design (vs v4's 145us harness / 148.7us TimelineSim):
  - psB (the 128 accumulating mean matmuls, ~27us of PE) is gone: the
    PSUM->SBUF transpose evacuations carry accum_out, producing per-
    (head, j-block) partial t-sums for free; a tiny DVE fold + 4
    single-column matmuls reconstruct Wm@mean.
  - the 8 (batch, head-quad) slots are software-pipelined: slot k's
    emission order is [transposes+evac, psA+tanh, meanMM], z(k-1),
    psW(k-1) 1st half, [tanhc, u4, logits], psW(k-1) 2nd half +
    outscale(k-1), [memsets, exp], load(k+2).  psW of the previous slot
    fills the PE while slot k's gate chain round-trips through ACT/DVE,
    and the deferred z never head-of-line-blocks the next transposes.
  - consts ride in 2 packed DMAs (one f32, one bf16); nat loads are
    per-hq (4 tiles, 2-slot prefetch ring); each batch's output leaves
    via ONE partition-strided DMA on the idle GPSIMD queue (a DMA on
    the SP queue would block the in-order load stream behind its wait).
  - outscales run on DVE (ACT is the busier ALU); evac routing is
    10/16 to DVE (KB_EVDVE); a 14-deep warm-matmul chain on a memset
    scratch tile starts at t~=0.8us so the PE clock is warm when the
    first nat tile lands (also needed on HW: the HAM clock governor
    ignores PE-transpose cycles).
  - transposes read the nat tiles through a stride-2 bf16 bitcast view
    (a f32's high uint16 IS its truncated bf16), so the PE transpose
    runs at 1.0 cyc/row instead of f32r's 1.5 with no downcast pass;
    psT/xt stay bf16 (DVE evacuations get the 2x 16-bit mode), and psW
    still streams the full-f32 nat tiles so output precision holds.
  - the LAST slot's exp runs as two 8-chunk waves into two zero-padded
    p tiles (p_a/p_b) with partial Z sums, so its psW first half gates
    only on wave A and covers wave B + z on the PE; its tanh(j3) —
    which only gates logits c12-15 — is deferred between the exp waves
    so it never queues on ACT ahead of tanhc/exp-A (KB_EXP2).
TimelineSim single-shot (the harness-tracked metric; sim matched the
v4 baseline 148.7 vs 145.0): 148.7 (v4) -> 127.4 (psB cut) -> 118.1
-> 117.6 (tail exp split + deferred tanh) -> 116.0 (PSUM rebalance
pst=4/psc=1, viable only after the ham/psC decoupling; tail evac
split re-tuned to 9/16 DVE under the new balance).
Engine busy/core: PE 85.3us (65k transpose + 65k psA + 65k psW cycles),
DMA 93.7 (now the pacer: the bus is gap-free from 2.9us to 96.5us),
ACT ~64, DVE ~44.  Span = lead-in 2 + bus 94 + last-slot serial chain
~13 + drain 1.7; the chain (mean->gates->softmax->psW of the final
slot) and the DMA lead-in are the remaining slack.  NOTE: GPSIMD
cannot access PSUM on real HW (BIR verifier) — the sim doesn't know
and will falsely promise wins from Pool evacuations/out-scales.
Numerics: rel err 2.13e-4 vs reference (bf16-trunc gate path is ~100x
inside the 2e-2 budget; f32r weighted sum).
"""

import os
import numpy as np

T, B, D = 2048, 32, 1024
H, N, K2 = 8, 128, 16
NCORES = 8
BL = B // NCORES          # 4 batches per core
TC = T // 128             # 16 t-chunks of 128
NQ = 4                    # nat tiles per (batch, head-quad); each holds 512 t
QW = 4 * N                # 512 cols per head-quad

LAST_RESULT = {}          # exec_time_ns etc. for test harness introspection


def _build(nc, tile, mybir, bass, whb_val, repeat=1, loop_n=0):
    f32 = mybir.dt.float32
    f32r = mybir.dt.float32r
    bf16 = mybir.dt.bfloat16
    AF = mybir.ActivationFunctionType

    hyp_s = nc.dram_tensor("hyp_s", [T, BL, D], f32, kind="ExternalInput").ap()
    cpack_d = nc.dram_tensor("cpack", [128, 233], f32,
                             kind="ExternalInput").ap()
    wgzm_d = nc.dram_tensor("wgzm", [N, 192], bf16, kind="ExternalInput").ap()
    out_s = nc.dram_tensor("out_s", [BL, D], f32, kind="ExternalOutput").ap()

    def r(ap):
        return ap.bitcast(f32r)

    with tile.TileContext(nc) as tc:
        from contextlib import ExitStack

        with ExitStack() as ctx:
            natf_b = int(os.environ.get("KB_NATF", "14"))
            xt_b = int(os.environ.get("KB_XT", "8"))
            g_b = int(os.environ.get("KB_G", "4"))
            # psc=1 became safe (and pst=4 profitable) once the warm
            # matmuls moved off psC and the tail restructure made all psC
            # column ranges interleave disjointly across consecutive slots
            pst_b = int(os.environ.get("KB_PST", "4"))
            psa_b = int(os.environ.get("KB_PSA", "2"))
            psc_b = int(os.environ.get("KB_PSC", "1"))
            psw_b = int(os.environ.get("KB_PSW", "1"))
            # number of the 16 per-slot PSUM->SBUF evacuations routed to
            # DVE (the rest go to ACT, which also carries tanh/exp/out)
            evdve = int(os.environ.get("KB_EVDVE", "8"))
            evdve_tail = int(os.environ.get("KB_EVDVET", "9"))
            # psW(k-1) chunk split around logits(k): first part covers the
            # u4 round-trip, the rest covers the exp round-trip
            psplit = int(os.environ.get("KB_PSPLIT", "8"))
            # NOTE: GPSIMD cannot access PSUM on real HW (neuronxcc BIR
            # verifier rejects it; the TimelineSim cost model doesn't know
            # and predicts a ~1.7us win from Pool out-scales).  All PSUM
            # readers must stay on DVE/ACT.
            osplit_mid = os.environ.get("KB_OSPLITM", "0") == "1"
            onpool = int(os.environ.get("KB_ONPOOL", "4"))
            evpool = os.environ.get("KB_EVPOOL", "0") == "1"
            # split the LAST slot's exp into two 8-chunk waves writing two
            # zero-padded p tiles, so its psW chunks c<8 gate only on wave
            # A and cover wave B + z on the PE (mid slots don't need it and
            # the extra 4 exp instrs would cost more ACT than they save)
            exp2_tail = os.environ.get("KB_EXP2", "1") == "1"
            cpool = ctx.enter_context(tc.tile_pool(name="consts", bufs=1))
            natf_pool = ctx.enter_context(tc.tile_pool(name="natf", bufs=natf_b))
            xt_pool = ctx.enter_context(tc.tile_pool(name="xt", bufs=xt_b))
            g_pool = ctx.enter_context(tc.tile_pool(name="g", bufs=g_b))
            sm_pool = ctx.enter_context(tc.tile_pool(name="small", bufs=4))
            out_pool = ctx.enter_context(tc.tile_pool(name="outp", bufs=1))
            pst_pool = ctx.enter_context(
                tc.tile_pool(name="pst", bufs=pst_b, space="PSUM"))
            psa_pool = ctx.enter_context(
                tc.tile_pool(name="psa", bufs=psa_b, space="PSUM"))
            psc_pool = ctx.enter_context(
                tc.tile_pool(name="psc", bufs=psc_b, space="PSUM"))
            psw_pool = ctx.enter_context(
                tc.tile_pool(name="psw", bufs=psw_b, space="PSUM"))

            cp = cpool.tile([128, 233], f32, tag="cpack")
            nc.sync.dma_start(r(cp[:]), r(cpack_d))
            wgzm = cpool.tile([N, 192], bf16, tag="wgzm")
            nc.sync.dma_start(wgzm[:], wgzm_d)
            ident = cp[:, 0:128]
            wbc = cp[:, 128:129]
            wmbc = cp[:, 129:130]
            whwm = cp[:, 130:134]
            ones_c = cp[:, 134:136]
            zmask = cp[:, 136:233]
            wgz = wgzm[:, 0:32]
            wmz = wgzm[:, 32:64]
            identb = wgzm[:, 64:192]
            # bf16 transposes read the HIGH 2 bytes of each f32 nat element
            # (little-endian: a f32's high uint16 IS its truncated bf16), so
            # the PE transpose runs at 1.0 cyc/row instead of f32r's 1.5 —
            # no downcast pass anywhere.  psW still streams the full-f32
            # nat tiles, so output precision is unchanged.
            tbf16 = os.environ.get("KB_TBF16", "1") == "1"
            whb_c = cpool.tile([128, 1], f32, tag="whb")
            nc.gpsimd.memset(whb_c[:], float(whb_val))

            # PE transposes don't register as PE-busy for the HAM clock
            # governor, so a transpose-heavy stretch can drop the PE to
            # 1.2GHz.  KB_WARM sprinkles tiny real matmuls to keep the
            # activity window fed.
            warm = os.environ.get("KB_WARM", "1") == "1"
            warm_n = int(os.environ.get("KB_WARMN", "14"))
            wide_early = int(os.environ.get("KB_WIDE", "2"))
            # scratch region in the psw bank for warm matmuls: partition 0,
            # cols 256:512 — disjoint from every outscale read slice
            # ([32q, 128q:128(q+1)]), and strictly PE-ordered vs psW writes.
            warm_ps = psw_pool.tile([128, 512], f32, tag="psw")

            def ham_warm(width=2):
                if warm:
                    nc.tensor.matmul(warm_ps[0:1, 256:256 + width],
                                     r(ones_c[:, 0:1]),
                                     r(ident[0:128, 0:width]),
                                     start=True, stop=True,
                                     skip_group_check=True)

            if warm:
                # pre-loop clock warm-up: chained dummies on a never-written
                # scratch tile (contents irrelevant), so they have NO input
                # dependencies and start right after the preamble barrier —
                # the PE is at full clock by the time the first nat tile
                # lands.
                wsc = cpool.tile([128, 128], f32, tag="wsc")
                nc.gpsimd.memset(wsc[:], 1.0)
                for _ in range(warm_n):
                    nc.tensor.matmul(warm_ps[0:1, 0:128],
                                     r(wsc[:, 0:1]), r(wsc[:]),
                                     start=True, stop=True,
                                     skip_group_check=True)

            out_sb = out_pool.tile([97, BL * D // 4], f32, tag="out")

            def load_nat_hq(b, hq, eng=None):
                tiles = []
                for j in range(NQ):
                    t0 = j * 4 * 128
                    nt = natf_pool.tile([128, 4 * QW], f32, tag="natf")
                    src = hyp_s[t0:t0 + 4 * 128, b:b + 1,
                                hq * QW:(hq + 1) * QW].rearrange(
                        "(c p) one d -> p c (one d)", p=128)
                    (eng or nc.sync).dma_start(
                        r(nt[:].rearrange("p (c d) -> p c d", c=4)),
                        r(src))
                    tiles.append(nt)
                return tiles

            def phase1a(natf, ev_n, wide=2, defer_tanh=False):
                """transposes+evac (with mean accum), psA+tanh, mean MMs."""
                psC = psc_pool.tile([128, 72], f32, tag="psc")
                msum = sm_pool.tile([128, 16], f32, tag="msum")
                xts = []
                ei = 0
                st_defer = None
                for j in range(NQ):
                    nf = natf[j]
                    xt = xt_pool.tile([128, 4 * QW], bf16, tag="xt")
                    for q in range(4):
                        psT = pst_pool.tile([128, 512],
                                            bf16 if tbf16 else f32,
                                            tag="pst")
                        ham_warm(wide)
                        for c in range(4):
                            if tbf16:
                                lo = 2 * (512 * c + 128 * q) + 1
                                nc.tensor.transpose(
                                    psT[:, 128 * c:128 * (c + 1)],
                                    nf[:].bitcast(bf16)[:, lo:lo + 255:2],
                                    identb[:])
                            else:
                                nc.tensor.transpose(
                                    r(psT[:, 128 * c:128 * (c + 1)]),
                                    r(nf[:, 512 * c + 128 * q:
                                         512 * c + 128 * (q + 1)]),
                                    r(ident[:]))
                        dst = xt[:, 512 * q:512 * (q + 1)]
                        acc = msum[:, 4 * q + j:4 * q + j + 1]
                        use_dve = (((ei + 1) * ev_n) // 16
                                   > (ei * ev_n) // 16)
                        use_pool = (not use_dve and evpool
                                    and (ei % 8) == 7)
                        ei += 1
                        if use_dve:
                            nc.vector.tensor_scalar(
                                dst, psT[:], 1.0, 0.0,
                                op0=mybir.AluOpType.mult,
                                op1=mybir.AluOpType.add,
                                accum_out=acc)
                        elif use_pool:
                            nc.gpsimd.tensor_scalar(
                                dst, psT[:], 1.0, 0.0,
                                op0=mybir.AluOpType.mult,
                                op1=mybir.AluOpType.add,
                                accum_out=acc)
                        else:
                            nc.scalar.activation(dst, psT[:], AF.Copy,
                                                 accum_out=acc)
                    xts.append(xt)

                g_sb = g_pool.tile([128, T], bf16, tag="g")
                for j in range(NQ):
                    psA = psa_pool.tile([128, 512], f32, tag="psa")
                    for q in range(4):
                        rhs = xts[j][:, 512 * q:512 * (q + 1)]
                        nc.tensor.matmul(
                            psA[32 * q:32 * q + 32, :], wgz[:], rhs,
                            start=True, stop=True,
                            tile_position=(0, 32 * q),
                            skip_group_check=True)
                    if defer_tanh and j == NQ - 1:
                        # last slot: tanh(j3) only gates logits c12-15, but
                        # in ACT's in-order queue it would sit AHEAD of
                        # tanhc/exp-wave-A and delay the psW-a path.  Defer
                        # its emission into phase1b, between the exp waves.
                        st_defer = (g_sb[:, 512 * j:512 * (j + 1)], psA)
                    else:
                        nc.scalar.activation(
                            g_sb[:, 512 * j:512 * (j + 1)], psA[:],
                            AF.Tanh, bias=wbc[:])

                # mean-gate path: fold the 4 j-partials per head, then 4
                # single-column matmuls put Wm@mean at partitions 32q+k
                # of psC[:, 68]
                m4 = sm_pool.tile([128, 4], bf16, tag="m4")
                with nc.allow_low_precision(
                        reason="4-elem j-partial fold; |sums|~45, bf16 ok"):
                    nc.vector.tensor_reduce(
                        m4[:], msum[:].rearrange("p (q j) -> p q j", q=4),
                        axis=mybir.AxisListType.X,
                        op=mybir.AluOpType.add)
                for q in range(4):
                    nc.tensor.matmul(
                        psC[32 * q:32 * q + 32, 68:69], wmz[:],
                        m4[:, q:q + 1],
                        start=True, stop=True,
                        tile_position=(0, 32 * q),
                        skip_group_check=True)
                return {"psC": psC, "g_sb": g_sb, "defer": st_defer}

            def zfin(st, z_home=None):
                """z matmul(s) + reciprocal (deferred to the next iteration
                so it never head-of-line blocks the next slot's transposes).
                z_home overrides the PSUM scratch columns — the deferred
                tail-first slot uses a LIVE slot's psC spare cols, since its
                own psC bank has been recycled by then."""
                bank = st["psC"] if z_home is None else z_home
                cs_ps = bank[0:128, 64:66]
                zz_ps = bank[0:97, 69:71]
                # Z without exp accum_out (saves 187ns of ACT accumulator
                # readout per exp instr on the serial ACT chain): a column-
                # sum matmul (p stationary, ones moving) puts per-column
                # sums at partitions 32q+c, a tiny DVE evac moves them to
                # SBUF, and a constant block-diagonal mask matmul lands
                # Z_q directly on partition 32q.  Emitted HERE (deferred):
                # the PE reaches this point with the exps long done, so
                # nothing head-of-line-blocks (a DVE reduce right after
                # the exps would stall the next slot's evacuations).
                prs = st["prs"]
                z_ps = cs_ps[0:97, 0:2]
                for i, pr in enumerate(prs):
                    nc.tensor.matmul(z_ps, r(pr[:]), r(ones_c[:]),
                                     start=(i == 0),
                                     stop=(i == len(prs) - 1),
                                     skip_group_check=True)
                zi_sb = sm_pool.tile([97, 1], f32, tag="zi_sb")
                nc.vector.reciprocal(zi_sb[:], z_ps[0:97, 0:1])
                st["zi_sb"] = zi_sb

            def phase1b(st, exp2=False):
                """tanhc, u4, logits, memsets, exp."""
                psC, g_sb = st["psC"], st["g_sb"]
                tanhc = sm_pool.tile([128, 1], f32, tag="tanhc")
                nc.scalar.activation(tanhc[:], psC[:, 68:69], AF.Tanh,
                                     bias=wmbc[:])
                u4 = sm_pool.tile([128, 4], bf16, tag="u4")
                nc.vector.tensor_mul(
                    u4[:], whwm[:], tanhc[:].broadcast_to([128, 4]))

                # logits t-major: l[t, q] = sum_p g[p, t] * U4[p, q]
                c_wave1 = TC // 2 if exp2 else TC
                for c in range(c_wave1):
                    nc.tensor.matmul(
                        psC[:, 4 * c:4 * c + 4],
                        g_sb[:, 128 * c:128 * (c + 1)], u4[:],
                        start=True, stop=True, skip_group_check=True)

                lview = psC[:, 0:64].rearrange("p (c q) -> p q c", q=4)
                if exp2:
                    # two zero-padded p tiles: p_a holds exp(l) for chunks
                    # c<8, p_b for c>=8 (zeros elsewhere).  psW chunk c<8
                    # then only depends on the first exp wave, so the final
                    # slot's psW first half overlaps the second wave.
                    p_a = sm_pool.tile([128, 144], f32, tag="p_a")
                    p_b = sm_pool.tile([128, 144], f32, tag="p_b")
                    pr_a = sm_pool.tile([128, 97], f32, tag="pr_a")
                    pr_b = sm_pool.tile([128, 97], f32, tag="pr_b")
                    nc.gpsimd.memset(p_a[:], 0.0)
                    nc.gpsimd.memset(p_b[:], 0.0)
                    nc.gpsimd.memset(pr_a[:], 1.0)
                    nc.gpsimd.memset(pr_b[:], 1.0)
                    hc = TC // 2
                    with nc.allow_low_precision(
                            reason="f32r accum is fp32-width"):
                        for h, (p_x, pr_x) in enumerate(
                                ((p_a, pr_a), (p_b, pr_b))):
                            # each wave reads an exactly-bounded psC view
                            # (cols 32h:32h+32), so wave A's strided read
                            # can never false-share with wave B's logits
                            # writes under span-based dependency tracking
                            lv_h = psC[:, 32 * h:32 * h + 32].rearrange(
                                "p (c q) -> p q c", q=4)
                            for q in range(4):
                                nc.scalar.activation(
                                    r(p_x[:, 32 * q + h * hc:
                                          32 * q + h * hc + hc]
                                      .unsqueeze(1)),
                                    lv_h[:, q:q + 1, :],
                                    AF.Exp, bias=whb_c[:],
                                    accum_out=r(pr_x[:, 32 * q:32 * q + 1]))
                            if h == 0:
                                # deferred tanh(j3) + second logits wave —
                                # emitted after exp wave A so the ACT queue
                                # reaches wave A without the tanh in front
                                if st.get("defer") is not None:
                                    g_dst, psA_d = st["defer"]
                                    nc.scalar.activation(
                                        g_dst, psA_d[:], AF.Tanh,
                                        bias=wbc[:])
                                for c in range(TC // 2, TC):
                                    nc.tensor.matmul(
                                        psC[:, 4 * c:4 * c + 4],
                                        g_sb[:, 128 * c:128 * (c + 1)],
                                        u4[:], start=True, stop=True,
                                        skip_group_check=True)
                    st["ps"] = (p_a, p_b)
                    st["nps"] = 2
                    st["prs"] = (pr_a, pr_b)
                else:
                    p_quad = sm_pool.tile([128, 144], f32, tag="p_quad")
                    pr_quad = sm_pool.tile([128, 97], f32, tag="pr_quad")
                    nc.gpsimd.memset(p_quad[:], 0.0)
                    nc.gpsimd.memset(pr_quad[:], 1.0)
                    with nc.allow_low_precision(
                            reason="f32r accum is fp32-width"):
                        for q in range(4):
                            nc.scalar.activation(
                                r(p_quad[:, 32 * q:32 * q + TC]
                                  .unsqueeze(1)),
                                lview[:, q:q + 1, :],
                                AF.Exp, bias=whb_c[:],
                                accum_out=r(pr_quad[:, 32 * q:32 * q + 1]))
                    st["ps"] = (p_quad, p_quad)
                    st["nps"] = 1
                    st["prs"] = (pr_quad,)

            def psw_mm(natf, st, c0, c1):
                """psW weighted-sum chunks [c0, c1)."""
                if c0 == 0:
                    psw_t = psw_pool.tile([128, 512], f32, tag="psw")
                    st["psW"] = psw_t
                psW = st["psW"]
                p_lo, p_hi = st["ps"]
                for c in range(c0, c1):
                    j, cl = c // 4, c % 4
                    p_x = p_lo if c < TC // 2 else p_hi
                    rhs = natf[j][:, 512 * cl:512 * (cl + 1)]
                    nc.tensor.matmul(psW[:], r(p_x[:, c:c + 128]),
                                     r(rhs),
                                     start=(c == 0), stop=(c == TC - 1),
                                     skip_group_check=True)

            def phase2fin(b, hq, st, split_eng=False, do_dma=None):
                """out scale (+ per-batch out DMA)."""
                psW, zi_sb = st["psW"], st["zi_sb"]
                for q in range(4):
                    col = b * (D // 4) + hq * N
                    if split_eng and q >= 2:
                        # at the kernel tail ACT is idle once exp-B drains
                        # (post exp2/defer restructure), so it can take two
                        # of the final out-scales in parallel with DVE and
                        # the out DMA launches ~0.7us sooner.  (GPSIMD
                        # cannot read PSUM, so Pool is not an option here.)
                        nc.scalar.activation(
                            out_sb[32 * q:32 * q + 1, col:col + N],
                            psW[32 * q:32 * q + 1, q * N:(q + 1) * N],
                            AF.Copy, bias=0.0,
                            scale=zi_sb[32 * q:32 * q + 1, 0:1])
                    else:
                        nc.vector.tensor_scalar(
                            out_sb[32 * q:32 * q + 1, col:col + N],
                            psW[32 * q:32 * q + 1, q * N:(q + 1) * N],
                            zi_sb[32 * q:32 * q + 1, 0:1], None,
                            op0=mybir.AluOpType.mult)
                if (hq == 1) if do_dma is None else do_dma:
                    # batch b fully scaled -> stream its row out now.  One
                    # partition-strided DMA, issued from the idle GPSIMD
                    # queue so it never blocks the SP load queue's head.
                    nc.gpsimd.dma_start(
                        out_s[b:b + 1, :].rearrange(
                            "one (j q n) -> one q j n", q=4, n=N),
                        out_sb[0:97:32,
                               b * (D // 4):(b + 1) * (D // 4)]
                        .rearrange("p (j n) -> p j n", n=N))

            def run_schedule(slots):
                n = len(slots)
                nat = {0: load_nat_hq(*slots[0])}
                if n > 1:
                    nat[1] = load_nat_hq(*slots[1])
                prev = None
                for i, (b, hq) in enumerate(slots):
                    st = phase1a(nat[i],
                                 evdve_tail if i == n - 1 else evdve,
                                 wide=wide_early if i < 2 else 2,
                                 defer_tanh=exp2_tail and i == n - 1)
                    if prev is not None:
                        pi, pst_ = prev
                        zfin(pst_)
                        psw_mm(nat[pi], pst_, 0, psplit)
                    phase1b(st, exp2=exp2_tail and i == n - 1)
                    if prev is not None:
                        pi, pst_ = prev
                        psw_mm(nat[pi], pst_, psplit, TC)
                        phase2fin(slots[pi][0], slots[pi][1], pst_,
                                  split_eng=osplit_mid)
                        del nat[pi]
                    prev = (i, st)
                    if i + 2 < n:
                        nat[i + 2] = load_nat_hq(*slots[i + 2])
                pi, pst_ = prev
                if pst_["nps"] == 2:
                    # final slot: psW's first half only needs the first exp
                    # wave; it covers the second wave + z on the PE
                    psw_mm(nat[pi], pst_, 0, TC // 2)
                    zfin(pst_)
                    psw_mm(nat[pi], pst_, TC // 2, TC)
                else:
                    zfin(pst_)
                    psw_mm(nat[pi], pst_, 0, TC)
                phase2fin(slots[pi][0], slots[pi][1], pst_,
                          split_eng=os.environ.get("KB_OSPLIT", "0") == "1")

            def run_schedule_tailfirst(slots):
                """The LAST slot's tiles load first and its whole gate
                pipeline runs during the DMA-starved head; its z/psW/out
                close out mid-kernel where bus-pacing hides the PE work.
                The kernel then ends on slots[n-2], whose chain is the only
                remaining post-last-byte work."""
                n = len(slots)
                last = n - 1
                nat = {last: load_nat_hq(*slots[last])}
                nat[0] = load_nat_hq(*slots[0])
                st_last = phase1a(nat[last], evdve, wide=wide_early)
                phase1b(st_last)
                nat[1] = load_nat_hq(*slots[1])
                prev = None
                for i in range(n - 1):
                    st = phase1a(nat[i],
                                 evdve_tail if i == n - 2 else evdve,
                                 wide=wide_early if i < 1 else 2)
                    if prev is not None:
                        pi, pst_ = prev
                        zfin(pst_)
                        psw_mm(nat[pi], pst_, 0, psplit)
                    phase1b(st)
                    if prev is not None:
                        pi, pst_ = prev
                        psw_mm(nat[pi], pst_, psplit, TC)
                        phase2fin(slots[pi][0], slots[pi][1], pst_,
                                  split_eng=osplit_mid)
                        del nat[pi]
                    prev = (i, st)
                    if i == 2:
                        # deferred close-out of the tail-first slot; its z
                        # lands in the LIVE slot's psC spare cols (its own
                        # bank has been recycled)
                        zfin(st_last, z_home=st["psC"][0:97, 69:71])
                        psw_mm(nat[last], st_last, 0, TC)
                        phase2fin(slots[last][0], slots[last][1], st_last,
                                  do_dma=False)
                        del nat[last]
                    if i + 2 < n - 1:
                        nat[i + 2] = load_nat_hq(*slots[i + 2])
                pi, pst_ = prev
                zfin(pst_)
                psw_mm(nat[pi], pst_, 0, TC)
                # slots[n-2] is (b3, hq0); its batch-mate (b3, hq1) was the
                # tail-first slot, already scaled -> DMA batch 3 now
                phase2fin(slots[pi][0], slots[pi][1], pst_, do_dma=True,
                          split_eng=os.environ.get("KB_OSPLIT", "0") == "1")

            base_slots = [(b, hq) for b in range(BL) for hq in range(2)]
            tailfirst = os.environ.get("KB_TAILFIRST", "0") == "1"
            if loop_n:
                with tc.For_i(0, loop_n, 1):
                    run_schedule(base_slots)
            elif tailfirst and repeat == 1:
                run_schedule_tailfirst(base_slots)
            else:
                run_schedule(base_slots * repeat)
    return nc


def _consts(inputs):
    import ml_dtypes
    W_w = np.asarray(inputs["W_w"], dtype=np.float32)      # (K2, N)
    W_b = np.asarray(inputs["W_b"], dtype=np.float32)      # (K2,)
    Wm_w = np.asarray(inputs["Wm_w"], dtype=np.float32)    # (K2, N)
    Wm_b = np.asarray(inputs["Wm_b"], dtype=np.float32)    # (K2,)
    Wh_w = np.asarray(inputs["Wh_w"], dtype=np.float32)    # (1, K2)

    bf = ml_dtypes.bfloat16
    wgz = np.zeros((N, 32), np.float32)
    wgz[:, 0:K2] = W_w.T
    wmz = np.zeros((N, 32), np.float32)
    wmz[:, 0:K2] = Wm_w.T / T
    wgzm = np.concatenate(
        [wgz, wmz, np.eye(N, dtype=np.float32)], axis=1).astype(bf)

    cpack = np.zeros((128, 233), np.float32)
    cpack[:, 0:128] = np.eye(128, dtype=np.float32)
    for q in range(4):
        cpack[32 * q:32 * q + K2, 128] = W_b
        cpack[32 * q:32 * q + K2, 129] = Wm_b
        cpack[32 * q:32 * q + K2, 130 + q] = Wh_w[0]
    cpack[:, 134:136] = 1.0
    # z block-diagonal mask (cols 136:233 = [128, 97]): row 32q+c (c<16)
    # has a 1 in col 32q, so  zmask.T @ colsums  lands Z_q on partition 32q
    for q in range(4):
        cpack[32 * q:32 * q + K2, 136 + 32 * q] = 1.0
    return {"cpack": cpack, "wgzm": wgzm}


def kernel(**inputs):
    import concourse.bass as bass
    import concourse.bacc as bacc
    import concourse.tile as tile
    import concourse.mybir as mybir
    from concourse import bass_utils

    hyp = np.ascontiguousarray(np.asarray(inputs["hyp"], dtype=np.float32))
    Wh_b = np.asarray(inputs["Wh_b"], dtype=np.float32)    # (1,)

    nc = bacc.Bacc("TRN2", target_bir_lowering=False, debug=False)
    _build(nc, tile, mybir, bass, float(Wh_b.reshape(-1)[0]))
    nc.compile()

    consts = _consts(inputs)
    in_maps = []
    for j in range(NCORES):
        m = {"hyp_s": np.ascontiguousarray(hyp[:, j * BL:(j + 1) * BL, :])}
        m.update(consts)
        in_maps.append(m)

    trace = os.environ.get("BASS_KERNEL_TRACE", "0") == "1"
    res = bass_utils.run_bass_kernel_spmd(
        nc, in_maps, core_ids=list(range(NCORES)), trace=trace)

    LAST_RESULT.clear()
    LAST_RESULT["exec_time_ns"] = res.exec_time_ns
    LAST_RESULT["trace"] = (res.instructions_and_trace[1]
                            if res.instructions_and_trace else None)
    LAST_RESULT["profile_json"] = res.profile_json

    out = np.concatenate([res.results[j]["out_s"] for j in range(NCORES)],
                         axis=0)
    return out.astype(np.float32)
